# revision 6
# baseline (speedup 1.0000x reference)
"""DGAT head (single attention head GAT) on 8 Trainium2 NeuronCores.

Strategy (row-sharded attention, per the sharding hint):
  - each core owns N/8 = 1024 query rows i of the [N, N] attention matrix,
  - adj is transposed + bf16-cast on the host so each core streams its
    [N, 1024] column-slice of adj.T with j (the softmax/contraction axis)
    on SBUF partitions.  That layout makes the softmax row-sum a matmul
    with a ones-column (TensorE contracts over partitions) and feeds the
    final attn @ h matmul directly -- no on-chip transpose, no 1x-rate
    tensor_reduce.
  - h (= x @ w), hl, hr (= h @ a1/a2) are tiny (0.1% of flops) and are
    precomputed on the host, replicated to all cores as matmul rhs.
  - masking (adj == 0 -> -9e15): softmax is computed unmasked on device
    (mathematically identical after normalization except for the masked
    terms, which for uniform adj are a handful of exact zeros); the host
    computes the exact correction for those entries and the device adds
    it to the PSUM accumulator before normalizing.
  - lrelu(a*adj + b): when a*adj+b >= 0 over the whole input (true for
    adj ~ U[0,1), a=b=1) lrelu is the identity affine, fused into ONE
    scalar_tensor_tensor op: m = (adjT + b/a) * (a*e).  A general path
    (explicit lrelu) exists as fallback.

Per-core main loop (16 iterations, 4 j-blocks of 128 per iteration):
  DMA [128, 4096] bf16 adjT tile -> DVE e = hl + hr (tensor_scalar, 4x)
  -> DVE m = (adjT + b/a)*(a e) (scalar_tensor_tensor, 2x)
  -> ACT expm = exp(m) -> PE psum[65, 1024] += [h|1]^T_j @ expm.
Epilogue: += correction, reciprocal of row sums (via a [128, 8] DRAM
bounce for lane parallelism), broadcast, multiply, elu, DMA out.
"""

import numpy as np
import ml_dtypes

N = 8192
D_IN = 128
D_OUT = 64
DP1 = D_OUT + 1
M_CORES = 8
NR = N // M_CORES  # 1024 query rows per core
SUB = 4            # j-blocks of 128 per main-loop iteration
NG = (N // 128) // SUB  # 16 main-loop iterations
JB = N // 128      # 64 j-blocks
NEG_SLOPE = 0.2

BF16 = ml_dtypes.bfloat16


def _lrelu_scalar(t: float) -> float:
    return t if t >= 0.0 else NEG_SLOPE * t


def _split_waits(nc, max_waits: int = 1):
    """This walrus build rejects instructions carrying more than ~2 sync
    waits.  Move excess waits onto same-engine NoOps inserted just before
    the over-limit instruction (the engine blocks on the NoOp's waits
    first, then issues the real instruction -- semantically identical)."""
    import concourse.mybir as mybir

    cnt = 0
    for fn in nc.m.functions:
        for bb in fn.blocks:
            out = []
            for inst in bb.instructions:
                si = inst.sync_info
                if si is not None and si.on_wait and len(si.on_wait) > max_waits:
                    waits = list(si.on_wait)
                    head, keep = waits[:-max_waits], waits[-max_waits:]
                    for i in range(0, len(head), max_waits):
                        nop = mybir.InstNoOp(
                            name=f"I-wsplit-{cnt}", engine=inst.engine
                        )
                        cnt += 1
                        nop.sync_info = mybir.SyncInfo(
                            on_wait=head[i : i + max_waits], on_update=[]
                        )
                        out.append(nop)
                    inst.sync_info = mybir.SyncInfo(
                        on_wait=keep, on_update=list(si.on_update or [])
                    )
                out.append(inst)
            bb.instructions[:] = out
    return nc


def build_nc(a: float, b: float, mode: str, exp_bias: float, reps: int = 1):
    """Build the SPMD Bass program (same program for all 8 cores).

    mode: 'affine'  -> lrelu(a*adj+b) == a*adj+b elementwise (host-checked);
                       e is pre-scaled by a on the host, m = (adjT + b/a)*e.
          'const'   -> a == 0: m = lrelu(b)*e with lrelu(b) folded into e.
          'general' -> explicit lrelu via max(v, NEG_SLOPE*v).
    exp_bias: softmax computes exp(m - exp_bias) on device; the host uses
    the same bias in the mask correction.  Cancels in normalization.
    """
    import concourse.bass as bass
    import concourse.mybir as mybir
    import concourse.tile as tile
    from concourse.vector_clock import ScopedClock
    from contextlib import ExitStack

    # Walrus's CTRL lowering rejects >2 sync waits on one instruction; the
    # stock TileContext tail drain collects one wait per logical processor.
    # Spread them across one nop each instead.
    def _drain_and_barrier(self, tick_clock, wait_clock):
        nc = self.nc
        vc = tick_clock.global_clock
        for proc in range(len(vc)):
            t = vc[proc]
            if t > 0:
                sc = ScopedClock()
                sc.require_at_least(None, proc, t)
                nop = nc.sync.nop()
                wait_clock.add_sem_waits(nop.ins, sc)
        nc.sync.drain()
        nc.all_engine_barrier()
        assert self.sems is not None
        popped = nc._tile_sem_poison_stack.pop()
        assert popped is self._sem_poison
        nc.clear_and_free_semaphores(list(self.sems.allocated().values()))
        nc.all_engine_barrier()

    tile.TileContext._drain_and_barrier = _drain_and_barrier

    dt = mybir.dt
    AF = mybir.ActivationFunctionType
    OP = mybir.AluOpType

    nc = bass.Bass()
    adjT = nc.dram_tensor("adjT", [N, NR], dt.float16, kind="ExternalInput")
    rhs = nc.dram_tensor("rhs", [N, DP1], dt.bfloat16, kind="ExternalInput")
    hlb = nc.dram_tensor("hlb", [128, SUB * NR], dt.float16, kind="ExternalInput")
    hrc = nc.dram_tensor("hrc", [128, JB], dt.float32, kind="ExternalInput")
    corrT = nc.dram_tensor("corrT", [DP1, NR], dt.float32, kind="ExternalInput")
    outT = nc.dram_tensor("outT", [D_OUT, NR], dt.float32, kind="ExternalOutput")

    with tile.TileContext(nc) as tc, ExitStack() as ctx:
        consts = ctx.enter_context(tc.tile_pool(name="consts", bufs=1))
        adjp = ctx.enter_context(tc.tile_pool(name="adjp", bufs=3))
        ep = ctx.enter_context(tc.tile_pool(name="ep", bufs=2))
        mp = ctx.enter_context(tc.tile_pool(name="mp", bufs=2))
        xp = ctx.enter_context(tc.tile_pool(name="xp", bufs=2))
        psum = ctx.enter_context(tc.tile_pool(name="psum", bufs=2, space="PSUM"))
        epi = ctx.enter_context(tc.tile_pool(name="epi", bufs=1))
        dsc = ctx.enter_context(tc.tile_pool(name="dsc", bufs=1, space="DRAM"))

        # ---- constants (loaded once) ----
        rhs_sb = consts.tile([128, JB * DP1], dt.bfloat16)
        nc.sync.dma_start(
            rhs_sb.rearrange("p (t d) -> p t d", d=DP1),
            rhs.rearrange("(t p) d -> p t d", p=128),
        )
        hlb_sb = consts.tile([128, SUB * NR], dt.float16)
        nc.sync.dma_start(hlb_sb[:], hlb[:])
        hrc_sb = consts.tile([128, JB], dt.float32)
        nc.sync.dma_start(hrc_sb[:], hrc[:])
        corr_sb = consts.tile([DP1, NR], dt.float32)
        nc.sync.dma_start(corr_sb[:], corrT[:])

        for _rep in range(reps):
            acc = psum.tile([DP1, NR], dt.float32)
            for g in range(NG):
                adj_sb = adjp.tile([128, SUB * NR], dt.float16)
                for s in range(SUB):
                    jb = SUB * g + s
                    nc.sync.dma_start(
                        adj_sb[:, s * NR : (s + 1) * NR],
                        adjT[jb * 128 : (jb + 1) * 128, :],
                    )
                e_sb = ep.tile([128, SUB * NR], dt.float16)
                for s in range(SUB):
                    jb = SUB * g + s
                    nc.vector.tensor_scalar_add(
                        e_sb[:, s * NR : (s + 1) * NR],
                        hlb_sb[:, s * NR : (s + 1) * NR],
                        hrc_sb[:, jb : jb + 1],
                    )
                m_sb = mp.tile([128, SUB * NR], dt.float32)
                if mode == "affine":
                    # m = (adjT + b/a) * (a*e); e already pre-scaled by a.
                    nc.vector.scalar_tensor_tensor(
                        m_sb[:], adj_sb[:], b / a, e_sb[:], OP.add, OP.mult
                    )
                elif mode == "const":
                    # a == 0: m = lrelu(b)*e, lrelu(b) folded into e on host.
                    nc.vector.tensor_copy(m_sb[:], e_sb[:])
                else:  # general lrelu
                    v_sb = mp.tile([128, SUB * NR], dt.float16, tag="v")
                    nc.vector.tensor_scalar(
                        v_sb[:], adj_sb[:], float(a), float(b), OP.mult, OP.add
                    )
                    l_sb = mp.tile([128, SUB * NR], dt.float16, tag="l")
                    nc.vector.scalar_tensor_tensor(
                        l_sb[:], v_sb[:], NEG_SLOPE, v_sb[:], OP.mult, OP.max
                    )
                    nc.vector.tensor_mul(m_sb[:], l_sb[:], e_sb[:])
                x_sb = xp.tile([128, SUB * NR], dt.bfloat16)
                nc.scalar.activation(
                    x_sb[:], m_sb[:], AF.Exp, bias=float(-exp_bias), scale=1.0
                )
                for s in range(SUB):
                    jb = SUB * g + s
                    lhsT = rhs_sb[:, jb * DP1 : (jb + 1) * DP1]
                    for hh in range(2):
                        nc.tensor.matmul(
                            acc[:, hh * 512 : (hh + 1) * 512],
                            lhsT,
                            x_sb[:, s * NR + hh * 512 : s * NR + (hh + 1) * 512],
                            start=(jb == 0),
                            stop=(jb == JB - 1),
                        )

            # ---- epilogue: correction, normalize, elu ----
            sT = epi.tile([DP1, NR], dt.float32)
            nc.vector.tensor_add(sT[:], acc[:], corr_sb[:])
            # reciprocal of the row sums; bounce via DRAM to spread the 1024
            # sums over 128 lanes (single-lane reciprocal would be ~8.5us)
            d1 = dsc.tile([1, NR], dt.float32)
            nc.sync.dma_start(d1[:], sT[D_OUT : D_OUT + 1, :])
            st8 = epi.tile([128, 8], dt.float32)
            nc.sync.dma_start(st8[:], d1.rearrange("o (q p) -> (o p) q", p=128))
            rt8 = epi.tile([128, 8], dt.float32)
            nc.vector.reciprocal(rt8[:], st8[:])
            d2 = dsc.tile([1, NR], dt.float32)
            nc.sync.dma_start(d2.rearrange("o (q p) -> (o p) q", p=128), rt8[:])
            rrep = epi.tile([D_OUT, NR], dt.float32)
            nc.sync.dma_start(rrep[:], d2[0:1, :].broadcast_to([D_OUT, NR]))
            hpT = epi.tile([D_OUT, NR], dt.float32)
            nc.vector.tensor_mul(hpT[:], sT[:D_OUT, :], rrep[:])
            # elu(x) = relu(x) + exp(min(x,0)) - 1
            neg = epi.tile([D_OUT, NR], dt.float32)
            nc.vector.tensor_scalar_min(neg[:], hpT[:], 0.0)
            ex = epi.tile([D_OUT, NR], dt.float32)
            nc.scalar.activation(ex[:], neg[:], AF.Exp)
            rel = epi.tile([D_OUT, NR], dt.float32)
            nc.vector.tensor_scalar_max(rel[:], hpT[:], 0.0)
            ot = epi.tile([D_OUT, NR], dt.float32)
            nc.vector.scalar_tensor_tensor(
                ot[:], ex[:], -1.0, rel[:], OP.add, OP.add
            )
            nc.sync.dma_start(outT[:], ot[:])

    return _split_waits(nc)


def _host_prep(input, adj, w, a, a_coeff, b_coeff):
    """Shard/layout prep on the host.  Returns (in_maps, a, b, mode, B)."""
    x = np.asarray(input, dtype=np.float32)[0].astype(np.float64)
    adj = np.asarray(adj, dtype=np.float32)
    w64 = np.asarray(w, dtype=np.float64)
    avec = np.asarray(a, dtype=np.float64).reshape(-1)
    af = float(np.asarray(a_coeff).reshape(-1)[0])
    bf = float(np.asarray(b_coeff).reshape(-1)[0])

    h = x @ w64                      # [N, 64]
    hl = h @ avec[:D_OUT]            # [N]
    hr = h @ avec[D_OUT:]            # [N]

    amin = float(adj.min())
    amax = float(adj.max())
    t_ends = (af * amin + bf, af * amax + bf)
    tmin, tmax = min(t_ends), max(t_ends)
    if af != 0.0 and tmin >= 0.0:
        mode = "affine"
    elif af == 0.0:
        mode = "const"
    else:
        mode = "general"

    l_ends = (_lrelu_scalar(tmin), _lrelu_scalar(tmax))
    e_ends = (
        hl.min() + hr.min(),
        hl.min() + hr.max(),
        hl.max() + hr.min(),
        hl.max() + hr.max(),
    )
    m_bound = max(abs(l * e) for l in l_ends for e in e_ends)
    B = max(0.0, float(m_bound) - 60.0)

    # pre-scale folded into e (hl/hr): 'affine' needs a*e; 'const' lrelu(b)*e
    if mode == "affine":
        pre = af
    elif mode == "const":
        pre = _lrelu_scalar(bf)
    else:
        pre = 1.0
    hl_s = hl * pre
    hr_s = hr * pre

    adjT_bf = adj.T.astype(np.float16)  # [N, N] fp16, C-contiguous
    h_bf = h.astype(np.float32).astype(BF16)
    rhs_np = np.concatenate(
        [h_bf, np.ones((N, 1), dtype=BF16)], axis=1
    )                                # [N, 65] bf16, replicated
    hrc_np = np.ascontiguousarray(
        hr_s.astype(np.float32).reshape(JB, 128).T
    )                                # [128, 64] f32, replicated

    l0 = _lrelu_scalar(bf)           # lrelu value at adj == 0
    in_maps = []
    for c in range(M_CORES):
        w0, w1 = c * NR, (c + 1) * NR
        adjT_c = np.ascontiguousarray(adjT_bf[:, w0:w1])
        hlw = hl_s[w0:w1].astype(np.float32).astype(np.float16)
        hlb_c = np.ascontiguousarray(
            np.tile(np.broadcast_to(hlw, (128, NR)), (1, SUB))
        )
        # exact mask correction for adj == 0 entries in this core's rows
        corr = np.zeros((DP1, NR), dtype=np.float64)
        zi, zj = np.nonzero(adj[w0:w1, :] == 0.0)
        if len(zi):
            mz = l0 * (hl[w0 + zi] + hr[zj])
            ev = np.exp(mz - B)
            acc_u = np.zeros((NR, D_OUT), dtype=np.float64)
            np.add.at(acc_u, zi, ev[:, None] * h[zj])
            acc_s = np.zeros(NR, dtype=np.float64)
            np.add.at(acc_s, zi, ev)
            corr[:D_OUT, :] = -acc_u.T
            corr[D_OUT, :] = -acc_s
        in_maps.append(
            {
                "adjT": adjT_c,
                "rhs": rhs_np,
                "hlb": hlb_c,
                "hrc": hrc_np,
                "corrT": corr.astype(np.float32),
            }
        )
    return in_maps, af, bf, mode, B


def kernel(input, adj, w, a, a_coeff, b_coeff):
    from concourse.bass_utils import run_bass_kernel_spmd

    in_maps, af, bf, mode, B = _host_prep(input, adj, w, a, a_coeff, b_coeff)
    nc = build_nc(af, bf, mode, B, reps=1)
    res = run_bass_kernel_spmd(nc, in_maps, list(range(M_CORES)))
    out = np.concatenate(
        [np.asarray(res.results[c]["outT"], dtype=np.float32).T for c in range(M_CORES)],
        axis=0,
    )
    return np.ascontiguousarray(out)
